# revision 1
# baseline (speedup 1.0000x reference)
"""Trainium2 Bass kernel for a 2-layer GraphConv + linear head (GCN-style).

Distribution: nodes (and their incident edges, by destination) are
partitioned across 8 NeuronCores. Weights are replicated. The per-layer
node-feature tables are exchanged with AllGather collectives.

Math (matches the reference):
    norm = clip(out_degree, 1)^-0.5           # per node, from src counts
    Y    = ((X * norm) @ w1)                  # layer1 matmul first (256>128)
    Z1   = segment_sum(Y[src] -> dst)
    H1   = relu(Z1 * norm + b1);  G = H1 * norm
    Z2   = segment_sum(G[src] -> dst)
    H2   = relu((Z2 @ w2) * norm + b2)
    OUT  = H2 @ w3.T + b3
    return (OUT, OUT)

On-device layout is feature-on-partition ("transposed") throughout, so
biases are per-partition scalars and per-node scale factors are a
[128, nodes] broadcast tile.

The scatter-add (segment_sum) runs on the TensorEngine:
  * base pass: every (dst, chunk) gets SLOTS fixed gather slots; a window
    of 128 tokens covers 128/SLOTS dsts and is reduced with a constant
    block-diagonal ones matrix as the moving operand. Pad slots gather a
    zeroed table row.
  * overflow pass: edges beyond the fixed slots use data-driven one-hot
    windows (iota-vs-dstloc is_equal on the VectorEngine).
PSUM accumulates per-element (first matmul start=True clears the bank).

Gathers use the custom SWDGE dma_gather instruction (int16 indices), so
the gather table is split into 4 address chunks (< 32768 rows each); each
rank's AllGather contribution carries trailing zero rows so every chunk
contains a zero row for padding tokens.
"""

import numpy as np

import concourse.bass as bass
import concourse.bacc as bacc
import concourse.tile as tile
import concourse.mybir as mybir
from concourse import bass_utils

F32 = mybir.dt.float32
I16 = mybir.dt.int16

NC_CORES = 8
NCHUNK = 4
SLOTS = 4          # base gather slots per (dst, chunk)
WIN = 128          # tokens per scatter window (PE contraction dim)
WPD = WIN // SLOTS # dsts covered by one base window


class Plan:
    """Host-side preprocessing: slot/overflow assignment, index arrays,
    static (shared-across-cores) schedule."""

    def __init__(self, n_nodes, e_subgraph, tile_d=512, zpad=44):
        N = n_nodes
        assert N % NC_CORES == 0
        self.N = N
        self.NLOC = N // NC_CORES
        self.ZPAD = zpad
        self.CONTRIB = self.NLOC + zpad
        assert (NC_CORES * self.CONTRIB) % NCHUNK == 0
        self.CHUNK = NC_CORES * self.CONTRIB // NCHUNK
        assert self.CHUNK <= 32767, self.CHUNK
        self.TILE_D = tile_d
        self.NT = -(-self.NLOC // tile_d)
        self.PADLOC = self.NT * tile_d

        src = np.asarray(e_subgraph[0], dtype=np.int64)
        dst = np.asarray(e_subgraph[1], dtype=np.int64)

        deg = np.bincount(src, minlength=N).astype(np.float32)
        self.norm = np.clip(deg, 1.0, None) ** -0.5

        srow = (src // self.NLOC) * self.CONTRIB + (src % self.NLOC)
        schunk = srow // self.CHUNK
        slidx = (srow - schunk * self.CHUNK).astype(np.int64)
        owner = dst // self.NLOC
        dloc = dst % self.NLOC

        # per-core edge assignment
        per_core = []
        for c in range(NC_CORES):
            sel = owner == c
            dl, ch, li = dloc[sel], schunk[sel], slidx[sel]
            order = np.lexsort((ch, dl))
            dl, ch, li = dl[order], ch[order], li[order]
            key = dl * NCHUNK + ch
            is_new = np.r_[True, key[1:] != key[:-1]] if len(key) else np.array([], bool)
            grp_id = np.cumsum(is_new) - 1 if len(key) else key
            if len(key):
                grp_start = np.flatnonzero(is_new)
                rank = np.arange(len(key)) - grp_start[grp_id]
            else:
                rank = key
            per_core.append((dl, ch, li, rank))

        zero_lidx = self.NLOC  # first zero row inside every chunk

        # base arrays + overflow lists
        NT, TILE_D, CHUNK = self.NT, self.TILE_D, self.CHUNK
        base = [np.full((NT, NCHUNK, TILE_D * SLOTS), zero_lidx, np.int64)
                for _ in range(NC_CORES)]
        ovf = [[[([], []) for _ in range(NCHUNK)] for _ in range(NT)]
               for _ in range(NC_CORES)]
        for c in range(NC_CORES):
            dl, ch, li, rank = per_core[c]
            t = dl // TILE_D
            din = dl - t * TILE_D
            bm = rank < SLOTS
            base[c][t[bm], ch[bm], din[bm] * SLOTS + rank[bm]] = li[bm]
            om = ~bm
            for tt, cc, dd, ll in zip(t[om], ch[om], din[om], li[om]):
                ovf[c][tt][cc][0].append(ll)
                ovf[c][tt][cc][1].append(dd)

        # static overflow window counts (max over cores)
        self.nw = np.zeros((NT, NCHUNK), np.int64)
        for t in range(NT):
            for cc in range(NCHUNK):
                mx = max(len(ovf[c][t][cc][0]) for c in range(NC_CORES))
                self.nw[t, cc] = -(-mx // WIN) if mx else 0

        # token stream: per tile, per chunk: [base TILE_D*SLOTS][ovf nw*WIN]
        self.seg = np.zeros((NT, NCHUNK), np.int64)
        for t in range(NT):
            for cc in range(NCHUNK):
                self.seg[t, cc] = TILE_D * SLOTS + self.nw[t, cc] * WIN
        self.tile_tokens = self.seg.sum(axis=1)
        self.tile_groups = self.tile_tokens // WIN
        self.tot_cols = int(self.tile_tokens.sum()) // 16
        self.nw_tot = int(self.nw.sum())

        # build per-core idx / dstloc arrays
        self.idx = np.zeros((NC_CORES, 128, self.tot_cols), np.int16)
        self.dstloc = np.full((NC_CORES, 128, max(self.nw_tot, 1)), -1.0, np.float32)
        for c in range(NC_CORES):
            col = 0
            w_i = 0
            for t in range(NT):
                for cc in range(NCHUNK):
                    toks = np.full(int(self.seg[t, cc]), zero_lidx, np.int64)
                    toks[:TILE_D * SLOTS] = base[c][t, cc]
                    ll, dd = ovf[c][t][cc]
                    if len(ll):
                        toks[TILE_D * SLOTS:TILE_D * SLOTS + len(ll)] = ll
                    seg = int(self.seg[t, cc])
                    wrapped = toks.astype(np.int16).reshape(seg // 16, 16).T
                    self.idx[c, :, col:col + seg // 16] = np.tile(wrapped, (8, 1))
                    col += seg // 16
                    for j in range(int(self.nw[t, cc])):
                        sl = dd[j * WIN:(j + 1) * WIN]
                        if len(sl):
                            self.dstloc[c, :len(sl), w_i] = sl
                        w_i += 1
            assert col == self.tot_cols

        # norm broadcast [128, PADLOC] per core
        self.normb = np.ones((NC_CORES, 128, self.PADLOC), np.float32)
        for c in range(NC_CORES):
            nl = self.norm[c * self.NLOC:(c + 1) * self.NLOC]
            self.normb[c, :, :self.NLOC] = nl[None, :]

    def consts(self):
        iota = np.broadcast_to(
            np.arange(self.TILE_D, dtype=np.float32), (128, self.TILE_D)).copy()
        onesb = np.zeros((128, WPD), np.float32)
        for tk in range(WIN):
            onesb[tk, tk // SLOTS] = 1.0
        return iota, onesb


def build_nc(plan: Plan, din, dh, dout):
    """Emit the bass program (shared SPMD across all cores)."""
    p = plan
    nc = bacc.Bacc("TRN2", target_bir_lowering=False, debug=False,
                   num_devices=NC_CORES)

    feats = nc.dram_tensor("feats", [p.NLOC, din], F32, kind="ExternalInput")
    w1_d = nc.dram_tensor("w1", [din, dh], F32, kind="ExternalInput")
    w2_d = nc.dram_tensor("w2", [dh, dh], F32, kind="ExternalInput")
    w3t_d = nc.dram_tensor("w3t", [dh, dout], F32, kind="ExternalInput")
    b1_d = nc.dram_tensor("b1", [dh, 1], F32, kind="ExternalInput")
    b2_d = nc.dram_tensor("b2", [dh, 1], F32, kind="ExternalInput")
    b3_d = nc.dram_tensor("b3", [dout, 1], F32, kind="ExternalInput")
    normb_d = nc.dram_tensor("normb", [128, p.PADLOC], F32, kind="ExternalInput")
    idx_d = nc.dram_tensor("idx", [128, p.tot_cols], I16, kind="ExternalInput")
    dstloc_d = nc.dram_tensor("dstloc", [128, max(p.nw_tot, 1)], F32,
                              kind="ExternalInput")
    iota_d = nc.dram_tensor("iota", [128, p.TILE_D], F32, kind="ExternalInput")
    onesb_d = nc.dram_tensor("onesb", [128, WPD], F32, kind="ExternalInput")
    out_d = nc.dram_tensor("outT", [dout, p.PADLOC], F32, kind="ExternalOutput")

    y_loc = nc.dram_tensor("y_loc", [p.CONTRIB, dh], F32)
    g_loc = nc.dram_tensor("g_loc", [p.CONTRIB, dh], F32)
    t_y = nc.dram_tensor("t_y", [NC_CORES * p.CONTRIB, dh], F32,
                         addr_space="Shared")
    t_g = nc.dram_tensor("t_g", [NC_CORES * p.CONTRIB, dh], F32,
                         addr_space="Shared")

    kt = din // 128  # K-tiles for layer-1 matmul
    gmax = int(p.tile_groups.max())
    cols_max = int(p.tile_tokens.max()) // 16

    with tile.TileContext(nc) as tc:
        with (
            tc.tile_pool(name="const", bufs=1) as cp,
            tc.tile_pool(name="xt", bufs=3) as xtp,
            tc.tile_pool(name="yt", bufs=3) as ytp,
            tc.tile_pool(name="nb", bufs=2) as nbp,
            tc.tile_pool(name="gath", bufs=2) as gp,
            tc.tile_pool(name="idxp", bufs=2) as ixp,
            tc.tile_pool(name="oh", bufs=2) as ohp,
            tc.tile_pool(name="mid", bufs=2) as midp,
            tc.tile_pool(name="psA", bufs=2, space="PSUM") as psA,
            tc.tile_pool(name="psB", bufs=2, space="PSUM") as psB,
            tc.tile_pool(name="psC", bufs=2, space="PSUM") as psC,
            tc.tile_pool(name="psD", bufs=2, space="PSUM") as psD,
        ):
            # ---- constants ----
            w1_sb = cp.tile([128, kt, dh], F32)
            for k in range(kt):
                nc.sync.dma_start(w1_sb[:, k, :], w1_d[k * 128:(k + 1) * 128, :])
            w2_sb = cp.tile([128, dh], F32)
            nc.sync.dma_start(w2_sb[:], w2_d[:, :])
            w3t_sb = cp.tile([128, dout], F32)
            nc.sync.dma_start(w3t_sb[:], w3t_d[:, :])
            b1_sb = cp.tile([dh, 1], F32)
            nc.sync.dma_start(b1_sb[:], b1_d[:, :])
            b2_sb = cp.tile([dh, 1], F32)
            nc.sync.dma_start(b2_sb[:], b2_d[:, :])
            b3_sb = cp.tile([dout, 1], F32)
            nc.sync.dma_start(b3_sb[:], b3_d[:, :])
            iota_sb = cp.tile([128, p.TILE_D], F32)
            nc.sync.dma_start(iota_sb[:], iota_d[:, :])
            onesb_sb = cp.tile([128, WPD], F32)
            nc.sync.dma_start(onesb_sb[:], onesb_d[:, :])
            dstloc_sb = cp.tile([128, max(p.nw_tot, 1)], F32)
            nc.sync.dma_start(dstloc_sb[:], dstloc_d[:, :])
            zeros_sb = cp.tile([128, dh], F32)
            nc.vector.memset(zeros_sb[:], 0.0)

            # ---- phase A: Y^T = w1^T-matmul over X^T tiles, scaled by norm ----
            for t in range(p.NT):
                r0 = t * p.TILE_D
                nr = min(p.TILE_D, p.NLOC - r0)
                if nr <= 0:
                    break
                ps = psA.tile([128, p.TILE_D], F32, space="PSUM")
                for k in range(kt):
                    xt = xtp.tile([128, p.TILE_D], F32)
                    src_ap = feats[r0:r0 + nr, k * 128:(k + 1) * 128] \
                        .rearrange("r f -> f r")
                    nc.sync.dma_start(xt[:, :nr], src_ap)
                    nc.tensor.matmul(ps[:, :nr], w1_sb[:, k, :], xt[:, :nr],
                                     start=(k == 0), stop=(k == kt - 1))
                nb = nbp.tile([128, p.TILE_D], F32)
                nc.sync.dma_start(nb[:], normb_d[:, r0:r0 + p.TILE_D])
                yt = ytp.tile([128, p.TILE_D], F32)
                nc.vector.tensor_tensor(out=yt[:, :nr], in0=ps[:, :nr],
                                        in1=nb[:, :nr],
                                        op=mybir.AluOpType.mult)
                nc.sync.dma_start(
                    y_loc[r0:r0 + nr, :].rearrange("r f -> f r"), yt[:, :nr])
            # zero pad rows of the contribution
            nc.sync.dma_start(y_loc[p.NLOC:p.CONTRIB, :], zeros_sb[:p.ZPAD, :])

            nc.gpsimd.collective_compute(
                "AllGather", mybir.AluOpType.bypass,
                ins=[y_loc.ap().opt()], outs=[t_y.ap().opt()],
                replica_groups=[list(range(NC_CORES))],
            )

            # ---- aggregation layers ----
            def agg_layer(table, layer):
                col0 = 0
                w_i0 = 0
                for t in range(p.NT):
                    d0 = t * p.TILE_D
                    nd = min(p.TILE_D, p.NLOC - d0)
                    cols_t = int(p.tile_tokens[t]) // 16
                    g_t = gp.tile([128, gmax, dh], F32, tag="gath")
                    ix = ixp.tile([128, cols_max], I16, tag="idx")
                    nc.sync.dma_start(ix[:, :cols_t], idx_d[:, col0:col0 + cols_t])
                    acc = psB.tile([128, p.TILE_D], F32, space="PSUM")

                    # gathers (one per chunk)
                    grp = 0
                    col = 0
                    spec = []  # (group, rhs_kind, info)
                    for cc in range(NCHUNK):
                        seg = int(p.seg[t, cc])
                        # HW packet limit: 64 descriptors/engine -> cap one
                        # gather instruction at 64*16 = 1024 tokens.
                        for off in range(0, seg, 1024):
                            sub = min(1024, seg - off)
                            nc.gpsimd.dma_gather(
                                g_t[:, grp + off // WIN:
                                    grp + (off + sub) // WIN, :],
                                table[cc * p.CHUNK:(cc + 1) * p.CHUNK, :],
                                ix[:, col + off // 16:col + (off + sub) // 16],
                                sub, sub, dh,
                            )
                        nbase = (p.TILE_D * SLOTS) // WIN
                        for w in range(nbase):
                            spec.append((grp + w, "base", w))
                        for j in range(int(p.nw[t, cc])):
                            spec.append((grp + nbase + j, "ovf", None))
                        grp += seg // WIN
                        col += seg // 16

                    w_i = w_i0
                    for si, (g, kind, info) in enumerate(spec):
                        start = si == 0
                        stop = si == len(spec) - 1
                        if kind == "base":
                            nc.tensor.matmul(
                                acc[:, info * WPD:(info + 1) * WPD],
                                g_t[:, g, :], onesb_sb[:],
                                start=start, stop=stop)
                        else:
                            oh = ohp.tile([128, p.TILE_D], F32, tag="oh")
                            nc.vector.tensor_scalar(
                                out=oh[:], in0=iota_sb[:],
                                scalar1=dstloc_sb[:, w_i:w_i + 1], scalar2=None,
                                op0=mybir.AluOpType.is_equal)
                            nc.tensor.matmul(acc[:], g_t[:, g, :], oh[:],
                                             start=start, stop=stop)
                            w_i += 1
                    w_i0 = w_i
                    col0 += cols_t

                    nb = nbp.tile([128, p.TILE_D], F32)
                    nc.sync.dma_start(nb[:], normb_d[:, d0:d0 + p.TILE_D])

                    if layer == 1:
                        # H1 = relu(acc*norm + b1); G = H1*norm -> g_loc
                        h = midp.tile([128, p.TILE_D], F32, tag="h")
                        nc.vector.tensor_tensor(out=h[:], in0=acc[:], in1=nb[:],
                                                op=mybir.AluOpType.mult)
                        hr = midp.tile([128, p.TILE_D], F32, tag="hr")
                        nc.scalar.activation(hr[:], h[:],
                                             mybir.ActivationFunctionType.Relu,
                                             bias=b1_sb[:, 0:1])
                        gt = ytp.tile([128, p.TILE_D], F32)
                        nc.vector.tensor_tensor(out=gt[:], in0=hr[:], in1=nb[:],
                                                op=mybir.AluOpType.mult)
                        if nd > 0:
                            nc.sync.dma_start(
                                g_loc[d0:d0 + nd, :].rearrange("r f -> f r"),
                                gt[:, :nd])
                    else:
                        # rst2 = (acc @ w2); H2 = relu(rst2*norm + b2)
                        a2 = midp.tile([128, p.TILE_D], F32, tag="a2")
                        nc.vector.tensor_copy(a2[:], acc[:])
                        ps2 = psC.tile([128, p.TILE_D], F32, space="PSUM")
                        nc.tensor.matmul(ps2[:], w2_sb[:], a2[:],
                                         start=True, stop=True)
                        h = midp.tile([128, p.TILE_D], F32, tag="h")
                        nc.vector.tensor_tensor(out=h[:], in0=ps2[:], in1=nb[:],
                                                op=mybir.AluOpType.mult)
                        h2 = midp.tile([128, p.TILE_D], F32, tag="hr")
                        nc.scalar.activation(h2[:], h[:],
                                             mybir.ActivationFunctionType.Relu,
                                             bias=b2_sb[:, 0:1])
                        ps3 = psD.tile([dout, p.TILE_D], F32, space="PSUM")
                        nc.tensor.matmul(ps3[:], w3t_sb[:], h2[:],
                                         start=True, stop=True)
                        ot = midp.tile([dout, p.TILE_D], F32, tag="ot")
                        nc.vector.tensor_scalar(
                            out=ot[:], in0=ps3[:], scalar1=b3_sb[:, 0:1],
                            scalar2=None, op0=mybir.AluOpType.add)
                        nc.sync.dma_start(out_d[:, d0:d0 + p.TILE_D], ot[:])

            agg_layer(t_y, layer=1)
            nc.sync.dma_start(g_loc[p.NLOC:p.CONTRIB, :], zeros_sb[:p.ZPAD, :])
            nc.gpsimd.collective_compute(
                "AllGather", mybir.AluOpType.bypass,
                ins=[g_loc.ap().opt()], outs=[t_g.ap().opt()],
                replica_groups=[list(range(NC_CORES))],
            )
            agg_layer(t_g, layer=2)

    nc.compile()
    return nc


def make_in_maps(plan: Plan, features, w1, b1, w2, b2, w3, b3):
    p = plan
    iota, onesb = p.consts()
    din = features.shape[1]
    in_maps = []
    for c in range(NC_CORES):
        in_maps.append(dict(
            feats=np.ascontiguousarray(
                features[c * p.NLOC:(c + 1) * p.NLOC]).astype(np.float32),
            w1=np.ascontiguousarray(w1, np.float32),
            w2=np.ascontiguousarray(w2, np.float32),
            w3t=np.ascontiguousarray(np.asarray(w3).T, np.float32),
            b1=np.asarray(b1, np.float32).reshape(-1, 1),
            b2=np.asarray(b2, np.float32).reshape(-1, 1),
            b3=np.asarray(b3, np.float32).reshape(-1, 1),
            normb=p.normb[c],
            idx=p.idx[c],
            dstloc=p.dstloc[c],
            iota=iota,
            onesb=onesb,
        ))
    return in_maps


def assemble_output(plan: Plan, results, dout):
    p = plan
    h = np.empty((p.N, dout), np.float32)
    for c in range(NC_CORES):
        h[c * p.NLOC:(c + 1) * p.NLOC] = results[c]["outT"][:, :p.NLOC].T
    return h


def run_graphconv(n_nodes, e_subgraph, features, w1, b1, w2, b2, w3, b3,
                  tile_d=512, mode="hw", trace=False):
    plan = Plan(n_nodes, e_subgraph, tile_d=tile_d)
    nc = build_nc(plan, features.shape[1], w1.shape[1], w3.shape[0])
    in_maps = make_in_maps(plan, features, w1, b1, w2, b2, w3, b3)
    if mode == "sim":
        from concourse import bass_interp
        sim = bass_interp.MultiCoreSim(nc, num_cores=NC_CORES)
        for c in range(NC_CORES):
            for k, v in in_maps[c].items():
                sim.cores[c].tensor(k)[:] = v
        sim.simulate(check_with_hw=False)
        results = [{"outT": sim.cores[c].mem_tensor("outT")}
                   for c in range(NC_CORES)]
        res = None
    else:
        res = bass_utils.run_bass_kernel_spmd(
            nc, in_maps, list(range(NC_CORES)), trace=trace)
        results = res.results
    h = assemble_output(plan, results, w3.shape[0])
    return h, res


def kernel(n_subgraph, e_subgraph, to_fetch, features, w1, b1, w2, b2, w3, b3):
    h, _ = run_graphconv(
        n_subgraph.shape[0], e_subgraph, features, w1, b1, w2, b2, w3, b3)
    return (h, h)



# revision 12
# speedup vs baseline: 7.6830x; 7.6830x over previous
"""Trainium2 Bass kernel for a 2-layer GraphConv + linear head (GCN-style).

Distribution: nodes (and their incident edges, by destination) are
partitioned across 8 NeuronCores. Weights are replicated. The per-layer
node-feature tables are exchanged with chunked AllGather collectives
(one chunk per contiguous tile range, so collectives overlap compute).

Math (matches the reference):
    norm = clip(out_degree, 1)^-0.5            # per node, from src counts
    Y    = (X @ w1) * norm                     # = ((X*norm) @ w1)
    Z1   = segment_sum(Y[src] -> dst)
    G    = relu(Z1 * norm + b1) * norm
    Z2   = segment_sum(G[src] -> dst)
    H2   = relu((Z2 @ w2) * norm + b2)
    OUT  = H2 @ w3.T + b3
    return (OUT, OUT)

Layout: node-major (node on partitions) everywhere except the final two
matmuls of layer 2, which need the dh contraction on partitions and get
it via on-chip PE transposes. Gather tables are bf16 [rows, 128] with
256-byte rows; row indices are int16 within 4 address chunks.

The scatter-add (segment_sum) runs on the TensorEngine with gathered
128-token windows as the MOVING operand and dst-major PSUM output
[dst, 4 blocks, dh]:
  * base pass: every (dst, chunk) has SLOTS fixed gather slots; the
    stationary operand is a constant block-diagonal ones matrix
    [128, 32]; each window hits a 32-partition slice. Pad slots gather
    a zero table row.
  * overflow pass: per (tile, chunk), extra edges sorted by dst; for
    each 128-dst block a window touches, a one-hot stationary matrix
    (VectorEngine is_equal of a per-block iota vs per-token dst) routes
    tokens to partitions. Untouched-core pad tokens carry dst -1.
"""

import numpy as np

import concourse.bass as bass
import concourse.bacc as bacc
import concourse.tile as tile
import concourse.mybir as mybir
from concourse import bass_utils

F32 = mybir.dt.float32
BF16 = mybir.dt.bfloat16
I16 = mybir.dt.int16

NC_CORES = 8
NCHUNK = 4
WIN = 128
SLOTS = 4
WPD = WIN // SLOTS
PADZ = 16          # zero rows appended to each chunk contribution
GATHER_MAX = 1024  # max tokens per dma_gather instruction


class Plan:
    """Host-side preprocessing: chunking, slot/overflow assignment,
    index arrays, static (shared-across-cores) schedule."""

    def __init__(self, n_nodes, e_subgraph, tile_d=512):
        N = n_nodes
        assert N % NC_CORES == 0
        self.N = N
        self.NLOC = N // NC_CORES
        self.TILE_D = tile_d
        self.NT = -(-self.NLOC // tile_d)
        self.PADLOC = self.NT * tile_d
        assert self.NT >= NCHUNK
        NT, TILE_D = self.NT, self.TILE_D
        NBLK = TILE_D // 128
        self.NBLK = NBLK

        # contiguous tile ranges per chunk (as even as possible)
        base_ct = self.NT // NCHUNK
        rem = self.NT - base_ct * NCHUNK
        self.ctiles = [base_ct + (1 if c < rem else 0) for c in range(NCHUNK)]
        self.ct0 = np.concatenate([[0], np.cumsum(self.ctiles)])  # tile idx
        self.crows = [ct * tile_d for ct in self.ctiles]          # data rows
        self.contrib = [r + PADZ for r in self.crows]             # + zeros
        self.trows = [8 * cr for cr in self.contrib]              # table rows
        assert all(tr <= 32767 for tr in self.trows), self.trows
        self.zero_idx = [self.crows[c] for c in range(NCHUNK)]    # rank0 pad
        tile2chunk = np.zeros(self.NT, np.int64)
        for c in range(NCHUNK):
            tile2chunk[self.ct0[c]:self.ct0[c + 1]] = c
        row0 = np.array([int(self.ct0[c]) * tile_d for c in range(NCHUNK)])

        src = np.asarray(e_subgraph[0], dtype=np.int64)
        dst = np.asarray(e_subgraph[1], dtype=np.int64)

        deg = np.bincount(src, minlength=N).astype(np.float32)
        self.norm = np.clip(deg, 1.0, None) ** -0.5

        # source -> (chunk, within-chunk row)
        rank_s = src // self.NLOC
        lloc = src % self.NLOC
        tile_s = lloc // tile_d
        schunk = tile2chunk[tile_s]
        srow = (rank_s * np.array(self.contrib)[schunk]
                + (lloc - row0[schunk]))
        owner = dst // self.NLOC
        dloc = dst % self.NLOC

        # per-core edge grouping and slot ranks (sorted by (dst, chunk))
        per_core = []
        for co in range(NC_CORES):
            sel = owner == co
            dl, ch, ro = dloc[sel], schunk[sel], srow[sel]
            order = np.lexsort((ro, ch, dl))
            dl, ch, ro = dl[order], ch[order], ro[order]
            key = dl * NCHUNK + ch
            if len(key):
                is_new = np.r_[True, key[1:] != key[:-1]]
                grp_start = np.flatnonzero(is_new)
                grp_id = np.cumsum(is_new) - 1
                rank = np.arange(len(key)) - grp_start[grp_id]
            else:
                rank = key
            per_core.append((dl, ch, ro, rank))

        # overflow counts: nov[t][c] = ceil(max over cores / WIN)
        cnt = np.zeros((NC_CORES, NT, NCHUNK), np.int64)
        for co in range(NC_CORES):
            dl, ch, ro, rank = per_core[co]
            om = rank >= SLOTS
            np.add.at(cnt[co], (dl[om] // TILE_D, ch[om]), 1)
        self.nov = -(-cnt.max(axis=0) // WIN)       # [NT, NCHUNK]
        self.nov_tot = int(self.nov.sum())

        # token stream layout: per (t, c): [base | ovf windows]
        self.seg = TILE_D * SLOTS + self.nov * WIN  # [NT, NCHUNK]
        self.tile_tokens = self.seg.sum(axis=1)
        self.tot_cols = int(self.tile_tokens.sum()) // 16
        self.gmax = int(self.seg.max() // WIN)      # groups per (t,c) max

        # build per-core idx / dstrel arrays + window blk unions
        self.idx = np.zeros((NC_CORES, 128, self.tot_cols), np.int16)
        self.dstrel = np.full((NC_CORES, 128, max(self.nov_tot, 1)), -1.0,
                              np.float32)
        # wblk[t][c][j] = union over cores of touched 128-dst blocks
        self.wblk = [[[set() for _ in range(int(self.nov[t, c]))]
                      for c in range(NCHUNK)] for t in range(NT)]
        for co in range(NC_CORES):
            dl, ch, ro, rank = per_core[co]
            t_all = dl // TILE_D
            din_all = dl % TILE_D
            col = 0
            w_i = 0
            for t in range(NT):
                for c in range(NCHUNK):
                    seg = int(self.seg[t, c])
                    toks = np.full(seg, self.zero_idx[c], np.int64)
                    m = (t_all == t) & (ch == c)
                    bm = m & (rank < SLOTS)
                    toks[din_all[bm] * SLOTS + rank[bm]] = ro[bm]
                    om = m & (rank >= SLOTS)
                    nw = int(self.nov[t, c])
                    if om.any():
                        # already sorted by dl
                        orow, odin = ro[om], din_all[om]
                        n = len(orow)
                        off = TILE_D * SLOTS
                        toks[off:off + n] = orow
                        for j in range(nw):
                            sl = odin[j * WIN:(j + 1) * WIN]
                            if len(sl):
                                self.dstrel[co, :len(sl), w_i + j] = sl
                                self.wblk[t][c][j].update(
                                    np.unique(sl // 128).tolist())
                    w_i += nw
                    wrapped = toks.astype(np.int16).reshape(seg // 16, 16).T
                    self.idx[co, :, col:col + seg // 16] = np.tile(wrapped,
                                                                   (8, 1))
                    col += seg // 16
            assert col == self.tot_cols
            assert w_i == self.nov_tot
        self.wblk = [[[sorted(b) for b in self.wblk[t][c]]
                      for c in range(NCHUNK)] for t in range(NT)]
        self.nmm_ovf = sum(len(b) for tt in self.wblk for cc in tt
                           for b in cc)

        # normcol [128, NT*NBLK]: column j = norm of local nodes
        # j*128 .. j*128+127 (pad nodes -> 1.0)
        ncols = self.PADLOC // 128
        self.normcol = np.ones((NC_CORES, 128, ncols), np.float32)
        for co in range(NC_CORES):
            nl = self.norm[co * self.NLOC:(co + 1) * self.NLOC]
            flat = np.ones(self.PADLOC, np.float32)
            flat[:self.NLOC] = nl
            self.normcol[co] = flat.reshape(ncols, 128).T

    def consts(self):
        onesb = np.zeros((128, WPD), np.float32)
        for tk in range(WIN):
            onesb[tk, tk // SLOTS] = 1.0
        iotas = np.stack([
            np.broadcast_to(np.arange(128 * b, 128 * (b + 1),
                                      dtype=np.float32), (128, 128))
            for b in range(self.NBLK)])
        ident = np.eye(128, dtype=np.float32)
        return onesb, iotas, ident


def build_nc(plan: Plan, din, dh, dout):
    """Emit the bass program (shared SPMD across all cores)."""
    p = plan
    NBLK = p.NBLK
    nc = bacc.Bacc("TRN2", target_bir_lowering=False, debug=False,
                   num_devices=NC_CORES, num_swdge_queues=4)

    feats = nc.dram_tensor("feats", [p.PADLOC, din], F32, kind="ExternalInput")
    w1_d = nc.dram_tensor("w1", [din, dh], BF16, kind="ExternalInput")
    w2_d = nc.dram_tensor("w2", [dh, dh], BF16, kind="ExternalInput")
    w3t_d = nc.dram_tensor("w3t", [dh, dout], BF16, kind="ExternalInput")
    b1b_d = nc.dram_tensor("b1b", [128, p.TILE_D], F32, kind="ExternalInput")
    b2_d = nc.dram_tensor("b2", [dh, 1], F32, kind="ExternalInput")
    b3_d = nc.dram_tensor("b3", [dout, 1], F32, kind="ExternalInput")
    normcol_d = nc.dram_tensor("normcol", [128, p.PADLOC // 128], F32,
                               kind="ExternalInput")
    idx_d = nc.dram_tensor("idx", [128, p.tot_cols], I16, kind="ExternalInput")
    dstrel_d = nc.dram_tensor("dstrel", [128, max(p.nov_tot, 1)], F32,
                              kind="ExternalInput")
    onesb_d = nc.dram_tensor("onesb", [128, WPD], BF16, kind="ExternalInput")
    iotas_d = nc.dram_tensor("iotas", [NBLK, 128, 128], F32,
                             kind="ExternalInput")
    ident_d = nc.dram_tensor("ident", [128, 128], F32, kind="ExternalInput")
    out_d = nc.dram_tensor("outT", [dout, p.PADLOC], F32,
                           kind="ExternalOutput")

    y_loc = [nc.dram_tensor(f"y_loc{c}", [p.contrib[c], dh], BF16)
             for c in range(NCHUNK)]
    g_loc = [nc.dram_tensor(f"g_loc{c}", [p.contrib[c], dh], BF16)
             for c in range(NCHUNK)]
    t_y = [nc.dram_tensor(f"t_y{c}", [p.trows[c], dh], BF16,
                          addr_space="Shared") for c in range(NCHUNK)]
    t_g = [nc.dram_tensor(f"t_g{c}", [p.trows[c], dh], BF16,
                          addr_space="Shared") for c in range(NCHUNK)]

    kt = din // 128
    NBLK_T = NBLK * kt // 2  # transposes per PSUM round in phase A
    groups = [list(range(NC_CORES))]
    # Gathers are the ONLY Pool-engine DMA instructions, so the tile
    # framework's DMASW semaphore lanes cycle (ctr % 8) in lockstep with
    # this counter; queue = lane % 4 keeps each lane on a fixed queue.
    qctr = [0]
    tile_chunk = [int(np.searchsorted(p.ct0, t, side="right")) - 1
                  for t in range(p.NT)]

    with tile.TileContext(nc) as tc:
        with (
            tc.tile_pool(name="const", bufs=1) as cp,
            tc.tile_pool(name="xt", bufs=2) as xtp,
            tc.tile_pool(name="ysb", bufs=2) as ysb,
            tc.tile_pool(name="gath", bufs=6) as gp,
            tc.tile_pool(name="oh", bufs=6) as ohp,
            tc.tile_pool(name="mid", bufs=2) as midp,
            tc.tile_pool(name="psAcc", bufs=2, space="PSUM") as psAcc,
            tc.tile_pool(name="psT", bufs=2, space="PSUM") as psT,
            tc.tile_pool(name="psR", bufs=2, space="PSUM") as psR,
            tc.tile_pool(name="psO", bufs=2, space="PSUM") as psO,
        ):
            # ---- constants (loads on scalar HWDGE queue) ----
            w1_sb = cp.tile([128, kt, dh], BF16)
            for k in range(kt):
                nc.scalar.dma_start(w1_sb[:, k, :],
                                    w1_d[k * 128:(k + 1) * 128, :])
            w2_sb = cp.tile([128, dh], BF16)
            nc.scalar.dma_start(w2_sb[:], w2_d[:, :])
            w3t_sb = cp.tile([128, dout], BF16)
            nc.scalar.dma_start(w3t_sb[:], w3t_d[:, :])
            b1b_sb = cp.tile([128, p.TILE_D], F32)
            nc.scalar.dma_start(b1b_sb[:], b1b_d[:, :])
            b2_sb = cp.tile([dh, 1], F32)
            nc.scalar.dma_start(b2_sb[:], b2_d[:, :])
            b3_sb = cp.tile([dout, 1], F32)
            nc.scalar.dma_start(b3_sb[:], b3_d[:, :])
            normcol_sb = cp.tile([128, p.PADLOC // 128], F32)
            nc.scalar.dma_start(normcol_sb[:], normcol_d[:, :])
            idx_sb = cp.tile([128, p.tot_cols], I16)
            nc.scalar.dma_start(idx_sb[:], idx_d[:, :])
            dstrel_sb = cp.tile([128, max(p.nov_tot, 1)], F32)
            nc.scalar.dma_start(dstrel_sb[:], dstrel_d[:, :])
            onesb_sb = cp.tile([128, WPD], BF16)
            nc.scalar.dma_start(onesb_sb[:], onesb_d[:, :])
            iotas_sb = cp.tile([128, NBLK, 128], F32)
            nc.scalar.dma_start(iotas_sb[:],
                                iotas_d[:, :, :].rearrange("b q f -> q b f"))
            ident_sb = cp.tile([128, 128], F32)
            nc.scalar.dma_start(ident_sb[:], ident_d[:, :])
            zeros_sb = cp.tile([PADZ, dh], BF16)
            nc.vector.memset(zeros_sb[:], 0.0)

            # pad-zero rows of every chunk contribution (y and g)
            for c in range(NCHUNK):
                nc.sync.dma_start(y_loc[c][p.crows[c]:p.contrib[c], :],
                                  zeros_sb[:])
                nc.sync.dma_start(g_loc[c][p.crows[c]:p.contrib[c], :],
                                  zeros_sb[:])

            # ---- phase A: Y = (X @ w1) * norm, node-major ----
            assert NBLK * kt == 2 * NBLK_T, (NBLK, kt)
            for t in range(p.NT):
                r0 = t * p.TILE_D
                ch = tile_chunk[t]
                xt = xtp.tile([128, NBLK, din], F32, tag="xt")
                nc.sync.dma_start(
                    xt[:],
                    feats[r0:r0 + p.TILE_D, :]
                    .rearrange("(b q) f -> q b f", q=128))
                xTs = midp.tile([128, NBLK * kt, 128], BF16, tag="xTs")
                for r in range(2):
                    xT = psT.tile([128, NBLK_T, 128], F32, space="PSUM",
                                  tag="psT")
                    for i in range(NBLK_T):
                        b, k = divmod(r * NBLK_T + i, kt)
                        nc.tensor.matmul(
                            xT[:, i, :],
                            xt[:, b, k * 128:(k + 1) * 128],
                            ident_sb[:], is_transpose=True,
                            start=(i == 0), stop=(i == NBLK_T - 1),
                            skip_group_check=True)
                    nc.scalar.activation(
                        xTs[:, r * NBLK_T:(r + 1) * NBLK_T, :]
                        .rearrange("q b f -> q (b f)"),
                        xT.rearrange("q b f -> q (b f)"),
                        mybir.ActivationFunctionType.Copy)
                yp = psAcc.tile([128, NBLK, dh], F32, space="PSUM", tag="acc")
                for b in range(NBLK):
                    for k in range(kt):
                        nc.tensor.matmul(
                            yp[:, b, :], xTs[:, b * kt + k, :], w1_sb[:, k, :],
                            start=(b == 0 and k == 0),
                            stop=(b == NBLK - 1 and k == kt - 1),
                            skip_group_check=True)
                ybs = ysb.tile([128, NBLK, dh], BF16, tag="ybs")
                for b in range(NBLK):
                    nc.scalar.activation(
                        ybs[:, b, :], yp[:, b, :],
                        mybir.ActivationFunctionType.Copy,
                        scale=normcol_sb[:, t * NBLK + b:t * NBLK + b + 1])
                lr0 = (t - int(p.ct0[ch])) * p.TILE_D
                nc.sync.dma_start(
                    y_loc[ch][lr0:lr0 + p.TILE_D, :]
                    .rearrange("(b q) f -> q b f", q=128),
                    ybs[:])
                if t == int(p.ct0[ch + 1]) - 1:
                    nc.gpsimd.collective_compute(
                        "AllGather", mybir.AluOpType.bypass,
                        ins=[y_loc[ch].ap().opt()],
                        outs=[t_y[ch].ap().opt()],
                        replica_groups=groups)

            # ---- aggregation layers ----
            col_ofs = np.zeros((p.NT, NCHUNK), np.int64)
            w_ofs = np.zeros((p.NT, NCHUNK), np.int64)
            acc_c = 0
            acc_w = 0
            for t in range(p.NT):
                for c in range(NCHUNK):
                    col_ofs[t, c] = acc_c
                    w_ofs[t, c] = acc_w
                    acc_c += int(p.seg[t, c]) // 16
                    acc_w += int(p.nov[t, c])

            def agg_tile(t, table):
                """Gathers + scatter matmuls for one dst tile -> psum acc."""
                acc = psAcc.tile([128, NBLK, dh], F32, space="PSUM",
                                 tag="acc")
                for c in range(NCHUNK):
                    seg = int(p.seg[t, c])
                    col0 = int(col_ofs[t, c])
                    gc = gp.tile([128, p.gmax, dh], BF16, tag="gath")
                    for off in range(0, seg, GATHER_MAX):
                        sub = min(GATHER_MAX, seg - off)
                        nc.gpsimd.dma_gather(
                            gc[:, off // WIN:(off + sub) // WIN, :],
                            table[c][:, :],
                            idx_sb[:, col0 + off // 16:
                                   col0 + (off + sub) // 16],
                            sub, sub, dh,
                            queue_num=(qctr[0] % 8) % 4)
                        qctr[0] += 1
                    # base windows (constant block-diagonal stationary)
                    for w in range(p.TILE_D * SLOTS // WIN):
                        d0 = w * WPD
                        nc.tensor.matmul(
                            acc[d0 % 128:d0 % 128 + WPD, d0 // 128, :],
                            onesb_sb[:], gc[:, w, :],
                            start=(c == 0 and d0 < 128),
                            stop=False, skip_group_check=True,
                            tile_position=(0, d0 % 128))
                    # overflow windows (one-hot per touched 128-dst block)
                    goff = p.TILE_D * SLOTS // WIN
                    for j in range(int(p.nov[t, c])):
                        w_i = int(w_ofs[t, c]) + j
                        for blk in p.wblk[t][c][j]:
                            oh = ohp.tile([128, 128], BF16, tag="oh")
                            nc.vector.tensor_scalar(
                                out=oh[:], in0=iotas_sb[:, blk, :],
                                scalar1=dstrel_sb[:, w_i:w_i + 1],
                                scalar2=None,
                                op0=mybir.AluOpType.is_equal)
                            nc.tensor.matmul(
                                acc[:, blk, :], oh[:], gc[:, goff + j, :],
                                start=False, stop=False,
                                skip_group_check=True)
                return acc

            # ---- layer 1 ----
            for t in range(p.NT):
                ch = tile_chunk[t]
                acc = agg_tile(t, t_y)
                # G = relu(acc*norm + b1)*norm -> bf16 g_loc
                t1 = midp.tile([128, NBLK, dh], F32, tag="t1")
                for b in range(NBLK):
                    nc.scalar.activation(
                        t1[:, b, :], acc[:, b, :],
                        mybir.ActivationFunctionType.Copy,
                        scale=normcol_sb[:, t * NBLK + b:t * NBLK + b + 1])
                t2 = midp.tile([128, NBLK, dh], F32, tag="t2")
                nc.vector.tensor_tensor(
                    out=t2.rearrange("q b f -> q (b f)"),
                    in0=t1.rearrange("q b f -> q (b f)"),
                    in1=b1b_sb[:], op=mybir.AluOpType.add)
                t3 = midp.tile([128, NBLK, dh], F32, tag="t3")
                nc.scalar.activation(
                    t3.rearrange("q b f -> q (b f)"),
                    t2.rearrange("q b f -> q (b f)"),
                    mybir.ActivationFunctionType.Relu)
                gb = ysb.tile([128, NBLK, dh], BF16, tag="ybs")
                for b in range(NBLK):
                    nc.scalar.activation(
                        gb[:, b, :], t3[:, b, :],
                        mybir.ActivationFunctionType.Copy,
                        scale=normcol_sb[:, t * NBLK + b:t * NBLK + b + 1])
                lr0 = (t - int(p.ct0[ch])) * p.TILE_D
                nc.sync.dma_start(
                    g_loc[ch][lr0:lr0 + p.TILE_D, :]
                    .rearrange("(b q) f -> q b f", q=128),
                    gb[:])
                if t == int(p.ct0[ch + 1]) - 1:
                    nc.gpsimd.collective_compute(
                        "AllGather", mybir.AluOpType.bypass,
                        ins=[g_loc[ch].ap().opt()],
                        outs=[t_g[ch].ap().opt()],
                        replica_groups=groups)

            # ---- layer 2 + head ----
            for t in range(p.NT):
                acc = agg_tile(t, t_g)
                # rst2T = w2.T @ (acc*norm).T ; H2 = relu(. + b2)
                # outT = w3t.T @ H2 + b3
                anT = midp.tile([128, NBLK, dh], F32, tag="anT")
                for b in range(NBLK):
                    nc.scalar.activation(
                        anT[:, b, :], acc[:, b, :],
                        mybir.ActivationFunctionType.Copy,
                        scale=normcol_sb[:, t * NBLK + b:t * NBLK + b + 1])
                zt = psT.tile([128, NBLK_T, 128], F32, space="PSUM",
                              tag="psT")
                assert NBLK_T == NBLK
                for b in range(NBLK):
                    nc.tensor.matmul(
                        zt[:, b, :], anT[:, b, :], ident_sb[:],
                        is_transpose=True, start=(b == 0),
                        stop=(b == NBLK - 1), skip_group_check=True)
                zts = midp.tile([128, NBLK, 128], BF16, tag="zts")
                nc.scalar.activation(
                    zts.rearrange("q b f -> q (b f)"),
                    zt.rearrange("q b f -> q (b f)"),
                    mybir.ActivationFunctionType.Copy)
                rst = psR.tile([128, p.TILE_D], F32, space="PSUM", tag="psR")
                nc.tensor.matmul(
                    rst[:], w2_sb[:], zts.rearrange("q b f -> q (b f)"),
                    start=True, stop=True, skip_group_check=True)
                h2 = midp.tile([128, p.TILE_D], BF16, tag="h2")
                nc.scalar.activation(
                    h2[:], rst[:], mybir.ActivationFunctionType.Relu,
                    bias=b2_sb[:, 0:1])
                o3 = psO.tile([dout, p.TILE_D], F32, space="PSUM", tag="psO")
                nc.tensor.matmul(o3[:], w3t_sb[:], h2[:],
                                 start=True, stop=True,
                                 skip_group_check=True)
                ot = midp.tile([dout, p.TILE_D], F32, tag="ot")
                nc.vector.tensor_scalar(
                    out=ot[:], in0=o3[:], scalar1=b3_sb[:, 0:1],
                    scalar2=None, op0=mybir.AluOpType.add)
                d0 = t * p.TILE_D
                nc.sync.dma_start(out_d[:, d0:d0 + p.TILE_D], ot[:])

    nc.compile()
    return nc


def _bf16(x):
    import ml_dtypes
    return np.ascontiguousarray(np.asarray(x, np.float32)).astype(
        ml_dtypes.bfloat16)


def make_in_maps(plan: Plan, features, w1, b1, w2, b2, w3, b3):
    p = plan
    din = features.shape[1]
    dh = w1.shape[1]
    onesb, iotas, ident = p.consts()
    b1b = np.tile(np.asarray(b1, np.float32)[None, :], (128, p.TILE_D // dh))
    in_maps = []
    for co in range(NC_CORES):
        fpad = np.zeros((p.PADLOC, din), np.float32)
        fpad[:p.NLOC] = np.asarray(features[co * p.NLOC:(co + 1) * p.NLOC],
                                   np.float32)
        in_maps.append(dict(
            feats=fpad,
            w1=_bf16(w1),
            w2=_bf16(w2),
            w3t=_bf16(np.asarray(w3).T),
            b1b=np.ascontiguousarray(b1b),
            b2=np.asarray(b2, np.float32).reshape(-1, 1),
            b3=np.asarray(b3, np.float32).reshape(-1, 1),
            normcol=p.normcol[co],
            idx=p.idx[co],
            dstrel=p.dstrel[co],
            onesb=_bf16(onesb),
            iotas=np.ascontiguousarray(iotas),
            ident=np.ascontiguousarray(ident),
        ))
    return in_maps


def assemble_output(plan: Plan, results, dout):
    p = plan
    h = np.empty((p.N, dout), np.float32)
    for co in range(NC_CORES):
        h[co * p.NLOC:(co + 1) * p.NLOC] = \
            np.asarray(results[co]["outT"], np.float32)[:, :p.NLOC].T
    return h


def run_graphconv(n_nodes, e_subgraph, features, w1, b1, w2, b2, w3, b3,
                  tile_d=512, mode="hw", trace=False):
    plan = Plan(n_nodes, e_subgraph, tile_d=tile_d)
    nc = build_nc(plan, features.shape[1], w1.shape[1], w3.shape[0])
    in_maps = make_in_maps(plan, features, w1, b1, w2, b2, w3, b3)
    if mode == "sim":
        from concourse import bass_interp
        sim = bass_interp.MultiCoreSim(nc, num_cores=NC_CORES)
        for co in range(NC_CORES):
            for k, v in in_maps[co].items():
                sim.cores[co].tensor(k)[:] = v
        sim.simulate(check_with_hw=False)
        results = [{"outT": sim.cores[co].mem_tensor("outT")}
                   for co in range(NC_CORES)]
        res = None
    else:
        res = bass_utils.run_bass_kernel_spmd(
            nc, in_maps, list(range(NC_CORES)), trace=trace)
        results = res.results
    h = assemble_output(plan, results, w3.shape[0])
    return h, res


def kernel(n_subgraph, e_subgraph, to_fetch, features, w1, b1, w2, b2, w3, b3):
    h, _ = run_graphconv(
        n_subgraph.shape[0], e_subgraph, features, w1, b1, w2, b2, w3, b3)
    return (h, h)
